# revision 23
# baseline (speedup 1.0000x reference)
"""CorrelationLayer (81-shift local correlation) on 8 Trainium2 NeuronCores.

Full inputs: feat1, feat2 [4, 128, 184, 320] fp32.
Full output: [4, 81, 184, 320] fp32,
  out[b, (dy+4)*9+(dx+4), y, x] = <f1n[b,:,y,x], f2n[b,:,y-dy,x-dx]>
  (features L2-normalized over C; f2 zero-padded outside the frame).

Sharding: 8 cores = batch(4) x W-halves(2).  Each core gets
  f1 shard [128, 184, 160] and f2 shard [128, 192, 168] (4-pixel
  zero-padded halo baked in on the host), both pre-cast to bf16 on the
  host (bf16 halves input HBM traffic and is scale-free, so raw
  correlations carry the same relative precision as normalized ones).

Per-core kernel — raw-correlation all-pairs matmuls ONLY:
  Per 8x16-pixel block, one PE matmul [C,128pix] x [C, 16x24 halo]
  -> PSUM [128, 384] all-pairs tile; PSUM pairs are evacuated to a
  per-band SBUF buffer by DVE/ACT/GpSimd in rotation (three-way split
  keeps each engine well under the DMA-ring floor); one [128, 3840]
  store per band, alternating between the two HWDGE rings (sync +
  scalar) which also carry the f2/f1 loads respectively.  Keeping the
  PE stream dense (no interleaved norm work, 4 PSUM pair-buffers)
  holds the PE at its warm 2.4 GHz clock.

L2 norms are NOT computed on device: the host computes fp32
inv-norms from the original inputs (cheap numpy reduction) and folds
them into the [81, H, W] gather it already performs during unshard.
"""

from contextlib import ExitStack

import numpy as np
import ml_dtypes

import concourse.bass as bass
import concourse.bacc as bacc
import concourse.tile as tile
from concourse import mybir
from concourse.bass_utils import run_bass_kernel_spmd

F32 = mybir.dt.float32
BF16 = mybir.dt.bfloat16

# problem constants (hardcoded per harness contract)
B, C, H, W = 4, 128, 184, 320
ROWS, WIDTH = 184, 160          # per-core shard (W-half)
PY, PX = 8, 16                  # pixel block
HY, HX = PY + 8, PX + 8         # halo block (16 x 24)
NHALO = HY * HX                 # 384
NBY, NBX = ROWS // PY, WIDTH // PX
NBLK = NBY * NBX                # 230
N1 = NBLK * 128                 # 29440 f1 pixels
ROWS2, W2 = ROWS + 8, WIDTH + 8
N2 = ROWS2 * W2                 # 32256 f2 pixels

NCH = 12                         # load chunks per input tensor
LA = 6                           # band lookahead for loads
SBANDS = 4                       # bands per store super-buffer
NSB = (NBY + SBANDS - 1) // SBANDS
SLC = 216                        # per-iy-group stored column slice

_compiled = {}


def _build_kernel(nc, f1, f2, out):
    tc_ctx = tile.TileContext(nc)
    with tc_ctx as tc, ExitStack() as ctx:
        ctx.enter_context(nc.allow_low_precision(
            reason="bf16 feature pipeline within correlation tolerance"))

        persist = ctx.enter_context(tc.tile_pool(name="persist", bufs=1))
        smpool = ctx.enter_context(tc.tile_pool(name="sm", bufs=4))
        psum_a = ctx.enter_context(
            tc.tile_pool(name="psum_a", bufs=4, space="PSUM"))

        f1b = persist.tile([C, N1], BF16)
        f2n = persist.tile([C, ROWS2, W2], BF16)

        # band-ordered load chunks: f1 on the scalar ring, f2 on the
        # sync ring (stores alternate between both; totals balance to
        # ~19 MB per ring).  First chunks are small so band 0's matmuls
        # start as early as possible.
        f1_bands = [0, 1]
        while f1_bands[-1] < NBY:
            f1_bands.append(min(NBY, f1_bands[-1] + 2))
        f2_rows = [0, 8, 16]
        while f2_rows[-1] < ROWS2:
            f2_rows.append(min(ROWS2, f2_rows[-1] + 16))

        state = {"f1": 0, "f2": 0}

        def ensure_f1(band_needed):
            while (state["f1"] < len(f1_bands) - 1
                   and f1_bands[state["f1"]] <= min(band_needed, NBY - 1)):
                g = state["f1"]
                c0 = f1_bands[g] * NBX * 128
                c1 = f1_bands[g + 1] * NBX * 128
                nc.scalar.dma_start(out=f1b[:, c0:c1], in_=f1[:, c0:c1])
                state["f1"] += 1

        def ensure_f2(row_needed):
            while (state["f2"] < len(f2_rows) - 1
                   and f2_rows[state["f2"]] <= min(row_needed, ROWS2 - 1)):
                g = state["f2"]
                r0, r1 = f2_rows[g], f2_rows[g + 1]
                nc.sync.dma_start(out=f2n[:, r0:r1], in_=f2[:, r0:r1])
                state["f2"] += 1

        ensure_f1(LA)
        ensure_f2((LA + 1) * PY + HY - 1)

        # pre-warm the PE clock (HAM): ~4 us of dummy matmuls into the
        # first PSUM pair while the first load chunks land, so the real
        # stream starts at the warm 2.4 GHz clock.
        warm = persist.tile([C, 512], BF16)
        nc.vector.memset(warm, 0.0)
        for _ in range(4):
            pw = psum_a.tile([128, 2, 512], F32, tag="ps")
            nc.tensor.matmul(pw[:, 0, :512], warm[:, :128], warm,
                             start=True, stop=True)
            nc.tensor.matmul(pw[:, 1, :512], warm[:, :128], warm,
                             start=True, stop=True)

        rot = 0
        for by in range(NBY):
            # prefetch ahead so the PE never starves (keeps HAM warm)
            ensure_f1(by + LA)
            ensure_f2((by + LA) * PY + HY - 1)
            sm = smpool.tile([128, NBX, NHALO], BF16)
            for bx0 in range(0, NBX, 2):
                pm = psum_a.tile([128, 2, 512], F32, tag="ps")
                for j in range(2):
                    bx = bx0 + j
                    blk = by * NBX + bx
                    lhsT = f1b[:, blk * 128:(blk + 1) * 128]
                    rhs = f2n[:, by * PY:by * PY + HY,
                              bx * PX:bx * PX + HX]
                    nc.tensor.matmul(pm[:, j, :NHALO], lhsT, rhs,
                                     start=True, stop=True)
                dst = sm[:, bx0:bx0 + 2, :]
                if rot == 0:
                    nc.vector.tensor_copy(out=dst, in_=pm[:, :, :NHALO])
                else:
                    nc.scalar.copy(out=dst, in_=pm[:, :, :NHALO])
                rot = (rot + 1) % 2
            eng = nc.sync if by % 2 == 0 else nc.scalar
            eng.dma_start(
                out=out[:, by * NBX * NHALO:(by + 1) * NBX * NHALO], in_=sm)


def _get_program():
    if "nc" not in _compiled:
        nc = bacc.Bacc("TRN2", target_bir_lowering=False, debug=False)
        f1 = nc.dram_tensor("f1", [C, N1], BF16,
                            kind="ExternalInput").ap()
        f2 = nc.dram_tensor("f2", [C, ROWS2, W2], BF16,
                            kind="ExternalInput").ap()
        out = nc.dram_tensor("tiles", [128, NBLK * NHALO], BF16,
                             kind="ExternalOutput").ap()
        _build_kernel(nc, f1, f2, out)
        nc.compile()
        _compiled["nc"] = nc
    return _compiled["nc"]


def _host_extract(tiles, inv1p, inv2p):
    """Sheared raw tiles [NBLK, 128, 384] + inv-norm planes ->
    [81, ROWS, WIDTH] normalized (fp32)."""
    v = tiles.reshape(NBY, NBX, PY, PX, HY, HX)
    out = np.empty((81, ROWS, WIDTH), np.float32)
    iy = np.arange(PY)[:, None]
    ix = np.arange(PX)[None, :]
    for dy in range(-4, 5):
        a = 4 - dy
        for dx in range(-4, 5):
            b = 4 - dx
            k = (dy + 4) * 9 + (dx + 4)
            g = v[:, :, iy, ix, iy + a, ix + b]      # [NBY, NBX, PY, PX]
            out[k] = (g.transpose(0, 2, 1, 3).reshape(ROWS, WIDTH)
                      * inv2p[a:a + ROWS, b:b + WIDTH])
    out *= inv1p[None]
    return out


def run_cores(in_maps, **kwargs):
    """Compile once and run the SPMD kernel on cores 0-7.

    Retries once: a freshly loaded NEFF occasionally hits a transient
    NRT exec-unit error right after a profiled session; the runtime
    recovers on the next execution.
    """
    import time

    nc = _get_program()
    try:
        return run_bass_kernel_spmd(nc, in_maps, core_ids=list(range(8)),
                                    **kwargs)
    except Exception:
        try:
            import jax.extend as jex

            jex.backend.clear_backends()
        except Exception:
            pass
        time.sleep(2.0)
        return run_bass_kernel_spmd(nc, in_maps, core_ids=list(range(8)),
                                    **kwargs)


def make_in_maps(feat1, feat2):
    feat1 = np.asarray(feat1, dtype=np.float32).astype(ml_dtypes.bfloat16)
    feat2 = np.asarray(feat2, dtype=np.float32).astype(ml_dtypes.bfloat16)
    in_maps = []
    for b in range(B):
        f2p = np.zeros((C, H + 8, W + 8), ml_dtypes.bfloat16)
        f2p[:, 4:-4, 4:-4] = feat2[b]
        for h in range(2):
            x0 = WIDTH * h
            # f1 block-major: [C, NBY, PY, NBX, PX] -> [C, NBY, NBX, PY, PX]
            f1s = feat1[b, :, :, x0:x0 + WIDTH].reshape(C, NBY, PY, NBX, PX)
            f1s = f1s.transpose(0, 1, 3, 2, 4).reshape(C, N1)
            in_maps.append({
                "f1": np.ascontiguousarray(f1s),
                "f2": np.ascontiguousarray(f2p[:, :, x0:x0 + WIDTH + 8]),
            })
    return in_maps


def _host_invnorms(feat1, feat2):
    """fp32 inverse L2 norms over C: inv1 [B, H, W]; inv2 padded
    [B, H+8, W+8] (zeros outside the frame -> raw dots there are 0)."""
    f1 = np.asarray(feat1, dtype=np.float32)
    f2 = np.asarray(feat2, dtype=np.float32)
    n1 = np.sqrt(np.einsum("bchw,bchw->bhw", f1, f1, optimize=True))
    n2 = np.sqrt(np.einsum("bchw,bchw->bhw", f2, f2, optimize=True))
    inv1 = 1.0 / np.maximum(n1, 1e-12)
    inv2 = np.zeros((B, H + 8, W + 8), np.float32)
    inv2[:, 4:-4, 4:-4] = 1.0 / np.maximum(n2, 1e-12)
    return inv1, inv2


def assemble(results, inv1, inv2):
    out = np.empty((B, 81, H, W), np.float32)
    for i, res in enumerate(results):
        tiles = np.asarray(res["tiles"]).astype(np.float32)
        tiles = tiles.reshape(128, NBLK, NHALO).transpose(1, 0, 2)
        b, h = i // 2, i % 2
        x0 = WIDTH * h
        out[b, :, :, x0:x0 + WIDTH] = _host_extract(
            tiles, inv1[b, :, x0:x0 + WIDTH],
            inv2[b, :, x0:x0 + WIDTH + 8])
    return out


def kernel(feat1, feat2):
    in_maps = make_in_maps(feat1, feat2)
    inv1, inv2 = _host_invnorms(feat1, feat2)
    res = run_cores(in_maps)
    return assemble(res.results, inv1, inv2)


# revision 24
# speedup vs baseline: 1.1246x; 1.1246x over previous
"""CorrelationLayer (81-shift local correlation) on 8 Trainium2 NeuronCores.

Full inputs: feat1, feat2 [4, 128, 184, 320] fp32.
Full output: [4, 81, 184, 320] fp32,
  out[b, (dy+4)*9+(dx+4), y, x] = <f1n[b,:,y,x], f2n[b,:,y-dy,x-dx]>
  (features L2-normalized over C; f2 zero-padded outside the frame).

Sharding: 8 cores = batch(4) x W-halves(2).  Each core gets
  f1 shard [128, 184, 160] and f2 shard [128, 192, 168] (4-pixel
  zero-padded halo baked in on the host), both pre-cast to bf16 on the
  host (bf16 halves input HBM traffic and is scale-free, so raw
  correlations carry the same relative precision as normalized ones).

Per-core kernel — raw-correlation all-pairs matmuls ONLY:
  Per 8x16-pixel block, one PE matmul [C,128pix] x [C, 16x24 halo]
  -> PSUM [128, 384] all-pairs tile; PSUM pairs are evacuated to a
  per-band SBUF buffer by DVE/ACT/GpSimd in rotation (three-way split
  keeps each engine well under the DMA-ring floor); one [128, 3840]
  store per band, alternating between the two HWDGE rings (sync +
  scalar) which also carry the f2/f1 loads respectively.  Keeping the
  PE stream dense (no interleaved norm work, 4 PSUM pair-buffers)
  holds the PE at its warm 2.4 GHz clock.

L2 norms are NOT computed on device: the host computes fp32
inv-norms from the original inputs (cheap numpy reduction) and folds
them into the [81, H, W] gather it already performs during unshard.
"""

from contextlib import ExitStack

import numpy as np
import ml_dtypes

import concourse.bass as bass
import concourse.bacc as bacc
import concourse.tile as tile
from concourse import mybir
from concourse.bass_utils import run_bass_kernel_spmd

F32 = mybir.dt.float32
BF16 = mybir.dt.bfloat16

# problem constants (hardcoded per harness contract)
B, C, H, W = 4, 128, 184, 320
ROWS, WIDTH = 184, 160          # per-core shard (W-half)
PY, PX = 8, 16                  # pixel block
HY, HX = PY + 8, PX + 8         # halo block (16 x 24)
NHALO = HY * HX                 # 384
NBY, NBX = ROWS // PY, WIDTH // PX
NBLK = NBY * NBX                # 230
N1 = NBLK * 128                 # 29440 f1 pixels
ROWS2, W2 = ROWS + 8, WIDTH + 8
N2 = ROWS2 * W2                 # 32256 f2 pixels

NCH = 12                         # load chunks per input tensor
LA = 6                           # band lookahead for loads
SBANDS = 4                       # bands per store super-buffer
NSB = (NBY + SBANDS - 1) // SBANDS
SLC = 216                        # per-iy-group stored column slice

_compiled = {}


def _build_kernel(nc, f1, f2, out):
    tc_ctx = tile.TileContext(nc)
    with tc_ctx as tc, ExitStack() as ctx:
        ctx.enter_context(nc.allow_low_precision(
            reason="bf16 feature pipeline within correlation tolerance"))

        persist = ctx.enter_context(tc.tile_pool(name="persist", bufs=1))
        smpool = ctx.enter_context(tc.tile_pool(name="sm", bufs=4))
        psum_a = ctx.enter_context(
            tc.tile_pool(name="psum_a", bufs=4, space="PSUM"))

        f1b = persist.tile([C, N1], BF16)
        f2n = persist.tile([C, ROWS2, W2], BF16)

        # band-ordered load chunks: f1 on the scalar ring, f2 on the
        # sync ring (stores alternate between both; totals balance to
        # ~19 MB per ring).  First chunks are small so band 0's matmuls
        # start as early as possible.
        f1_bands = [0]
        while f1_bands[-1] < NBY:
            f1_bands.append(min(NBY, f1_bands[-1] + 3))
        f2_rows = [0]
        while f2_rows[-1] < ROWS2:
            f2_rows.append(min(ROWS2, f2_rows[-1] + 24))

        state = {"f1": 0, "f2": 0}

        def ensure_f1(band_needed):
            while (state["f1"] < len(f1_bands) - 1
                   and f1_bands[state["f1"]] <= min(band_needed, NBY - 1)):
                g = state["f1"]
                c0 = f1_bands[g] * NBX * 128
                c1 = f1_bands[g + 1] * NBX * 128
                nc.scalar.dma_start(out=f1b[:, c0:c1], in_=f1[:, c0:c1])
                state["f1"] += 1

        def ensure_f2(row_needed):
            while (state["f2"] < len(f2_rows) - 1
                   and f2_rows[state["f2"]] <= min(row_needed, ROWS2 - 1)):
                g = state["f2"]
                r0, r1 = f2_rows[g], f2_rows[g + 1]
                nc.sync.dma_start(out=f2n[:, r0:r1], in_=f2[:, r0:r1])
                state["f2"] += 1

        ensure_f1(LA)
        ensure_f2((LA + 1) * PY + HY - 1)

        # pre-warm the PE clock (HAM): ~4 us of dummy matmuls into the
        # first PSUM pair while the first load chunks land, so the real
        # stream starts at the warm 2.4 GHz clock.
        warm = persist.tile([C, 512], BF16)
        nc.vector.memset(warm, 0.0)
        for _ in range(4):
            pw = psum_a.tile([128, 2, 512], F32, tag="ps")
            nc.tensor.matmul(pw[:, 0, :512], warm[:, :128], warm,
                             start=True, stop=True)
            nc.tensor.matmul(pw[:, 1, :512], warm[:, :128], warm,
                             start=True, stop=True)

        rot = 0
        for by in range(NBY):
            # prefetch ahead so the PE never starves (keeps HAM warm)
            ensure_f1(by + LA)
            ensure_f2((by + LA) * PY + HY - 1)
            sm = smpool.tile([128, NBX, NHALO], BF16)
            for bx0 in range(0, NBX, 2):
                pm = psum_a.tile([128, 2, 512], F32, tag="ps")
                for j in range(2):
                    bx = bx0 + j
                    blk = by * NBX + bx
                    lhsT = f1b[:, blk * 128:(blk + 1) * 128]
                    rhs = f2n[:, by * PY:by * PY + HY,
                              bx * PX:bx * PX + HX]
                    nc.tensor.matmul(pm[:, j, :NHALO], lhsT, rhs,
                                     start=True, stop=True)
                dst = sm[:, bx0:bx0 + 2, :]
                if rot == 0:
                    nc.vector.tensor_copy(out=dst, in_=pm[:, :, :NHALO])
                else:
                    nc.scalar.copy(out=dst, in_=pm[:, :, :NHALO])
                rot = (rot + 1) % 2
            eng = nc.sync if by % 2 == 0 else nc.scalar
            eng.dma_start(
                out=out[:, by * NBX * NHALO:(by + 1) * NBX * NHALO], in_=sm)


def _get_program():
    if "nc" not in _compiled:
        nc = bacc.Bacc("TRN2", target_bir_lowering=False, debug=False)
        f1 = nc.dram_tensor("f1", [C, N1], BF16,
                            kind="ExternalInput").ap()
        f2 = nc.dram_tensor("f2", [C, ROWS2, W2], BF16,
                            kind="ExternalInput").ap()
        out = nc.dram_tensor("tiles", [128, NBLK * NHALO], BF16,
                             kind="ExternalOutput").ap()
        _build_kernel(nc, f1, f2, out)
        nc.compile()
        _compiled["nc"] = nc
    return _compiled["nc"]


def _host_extract(tiles, inv1p, inv2p):
    """Sheared raw tiles [NBLK, 128, 384] + inv-norm planes ->
    [81, ROWS, WIDTH] normalized (fp32)."""
    v = tiles.reshape(NBY, NBX, PY, PX, HY, HX)
    out = np.empty((81, ROWS, WIDTH), np.float32)
    iy = np.arange(PY)[:, None]
    ix = np.arange(PX)[None, :]
    for dy in range(-4, 5):
        a = 4 - dy
        for dx in range(-4, 5):
            b = 4 - dx
            k = (dy + 4) * 9 + (dx + 4)
            g = v[:, :, iy, ix, iy + a, ix + b]      # [NBY, NBX, PY, PX]
            out[k] = (g.transpose(0, 2, 1, 3).reshape(ROWS, WIDTH)
                      * inv2p[a:a + ROWS, b:b + WIDTH])
    out *= inv1p[None]
    return out


def run_cores(in_maps, **kwargs):
    """Compile once and run the SPMD kernel on cores 0-7.

    Retries once: a freshly loaded NEFF occasionally hits a transient
    NRT exec-unit error right after a profiled session; the runtime
    recovers on the next execution.
    """
    import time

    nc = _get_program()
    try:
        return run_bass_kernel_spmd(nc, in_maps, core_ids=list(range(8)),
                                    **kwargs)
    except Exception:
        try:
            import jax.extend as jex

            jex.backend.clear_backends()
        except Exception:
            pass
        time.sleep(2.0)
        return run_bass_kernel_spmd(nc, in_maps, core_ids=list(range(8)),
                                    **kwargs)


def make_in_maps(feat1, feat2):
    feat1 = np.asarray(feat1, dtype=np.float32).astype(ml_dtypes.bfloat16)
    feat2 = np.asarray(feat2, dtype=np.float32).astype(ml_dtypes.bfloat16)
    in_maps = []
    for b in range(B):
        f2p = np.zeros((C, H + 8, W + 8), ml_dtypes.bfloat16)
        f2p[:, 4:-4, 4:-4] = feat2[b]
        for h in range(2):
            x0 = WIDTH * h
            # f1 block-major: [C, NBY, PY, NBX, PX] -> [C, NBY, NBX, PY, PX]
            f1s = feat1[b, :, :, x0:x0 + WIDTH].reshape(C, NBY, PY, NBX, PX)
            f1s = f1s.transpose(0, 1, 3, 2, 4).reshape(C, N1)
            in_maps.append({
                "f1": np.ascontiguousarray(f1s),
                "f2": np.ascontiguousarray(f2p[:, :, x0:x0 + WIDTH + 8]),
            })
    return in_maps


def _host_invnorms(feat1, feat2):
    """fp32 inverse L2 norms over C: inv1 [B, H, W]; inv2 padded
    [B, H+8, W+8] (zeros outside the frame -> raw dots there are 0)."""
    f1 = np.asarray(feat1, dtype=np.float32)
    f2 = np.asarray(feat2, dtype=np.float32)
    n1 = np.sqrt(np.einsum("bchw,bchw->bhw", f1, f1, optimize=True))
    n2 = np.sqrt(np.einsum("bchw,bchw->bhw", f2, f2, optimize=True))
    inv1 = 1.0 / np.maximum(n1, 1e-12)
    inv2 = np.zeros((B, H + 8, W + 8), np.float32)
    inv2[:, 4:-4, 4:-4] = 1.0 / np.maximum(n2, 1e-12)
    return inv1, inv2


def assemble(results, inv1, inv2):
    out = np.empty((B, 81, H, W), np.float32)
    for i, res in enumerate(results):
        tiles = np.asarray(res["tiles"]).astype(np.float32)
        tiles = tiles.reshape(128, NBLK, NHALO).transpose(1, 0, 2)
        b, h = i // 2, i % 2
        x0 = WIDTH * h
        out[b, :, :, x0:x0 + WIDTH] = _host_extract(
            tiles, inv1[b, :, x0:x0 + WIDTH],
            inv2[b, :, x0:x0 + WIDTH + 8])
    return out


def kernel(feat1, feat2):
    in_maps = make_in_maps(feat1, feat2)
    inv1, inv2 = _host_invnorms(feat1, feat2)
    res = run_cores(in_maps)
    return assemble(res.results, inv1, inv2)


# revision 28
# speedup vs baseline: 1.2089x; 1.0750x over previous
"""CorrelationLayer (81-shift local correlation) on 8 Trainium2 NeuronCores.

Full inputs: feat1, feat2 [4, 128, 184, 320] fp32.
Full output: [4, 81, 184, 320] fp32,
  out[b, (dy+4)*9+(dx+4), y, x] = <f1n[b,:,y,x], f2n[b,:,y-dy,x-dx]>
  (features L2-normalized over C; f2 zero-padded outside the frame).

Sharding: 8 cores = batch(4) x W-halves(2).  Each core gets
  f1 shard [128, 184, 160] and f2 shard [128, 192, 168] (4-pixel
  zero-padded halo baked in on the host), both pre-cast to bf16 on the
  host (bf16 halves input HBM traffic and is scale-free, so raw
  correlations carry the same relative precision as normalized ones).

Per-core kernel — raw-correlation all-pairs matmuls ONLY:
  Per 8x16-pixel block, one PE matmul [C,128pix] x [C, 16x24 halo]
  -> PSUM [128, 384] all-pairs tile; PSUM pairs are evacuated to a
  per-band SBUF buffer by DVE/ACT/GpSimd in rotation (three-way split
  keeps each engine well under the DMA-ring floor); one [128, 3840]
  store per band, alternating between the two HWDGE rings (sync +
  scalar) which also carry the f2/f1 loads respectively.  Keeping the
  PE stream dense (no interleaved norm work, 4 PSUM pair-buffers)
  holds the PE at its warm 2.4 GHz clock.

L2 norms are NOT computed on device: the host computes fp32
inv-norms from the original inputs (cheap numpy reduction) and folds
them into the [81, H, W] gather it already performs during unshard.
"""

from contextlib import ExitStack

import numpy as np
import ml_dtypes

import concourse.bass as bass
import concourse.bacc as bacc
import concourse.tile as tile
from concourse import mybir
from concourse.bass_utils import run_bass_kernel_spmd

F32 = mybir.dt.float32
BF16 = mybir.dt.bfloat16

# problem constants (hardcoded per harness contract)
B, C, H, W = 4, 128, 184, 320
ROWS, WIDTH = 184, 160          # per-core shard (W-half)
PY, PX = 8, 16                  # pixel block
HY, HX = PY + 8, PX + 8         # halo block (16 x 24)
NHALO = HY * HX                 # 384
NBY, NBX = ROWS // PY, WIDTH // PX
NBLK = NBY * NBX                # 230
N1 = NBLK * 128                 # 29440 f1 pixels
ROWS2, W2 = ROWS + 8, WIDTH + 8
N2 = ROWS2 * W2                 # 32256 f2 pixels

NCH = 12                         # load chunks per input tensor
LA = 6                           # band lookahead for loads
SBANDS = 4                       # bands per store super-buffer
NSB = (NBY + SBANDS - 1) // SBANDS
SLC = 216                        # per-iy-group stored column slice

_compiled = {}


def _build_kernel(nc, f1, f2, out):
    tc_ctx = tile.TileContext(nc)
    with tc_ctx as tc, ExitStack() as ctx:
        ctx.enter_context(nc.allow_low_precision(
            reason="bf16 feature pipeline within correlation tolerance"))

        persist = ctx.enter_context(tc.tile_pool(name="persist", bufs=1))
        smpool = ctx.enter_context(tc.tile_pool(name="sm", bufs=4))
        psum_a = ctx.enter_context(
            tc.tile_pool(name="psum_a", bufs=4, space="PSUM"))

        f1b = persist.tile([C, N1], BF16)
        f2n = persist.tile([C, ROWS2, W2], BF16)

        # band-ordered load chunks: f1 on the scalar ring, f2 on the
        # sync ring (stores alternate between both; totals balance to
        # ~19 MB per ring).  First chunks are small so band 0's matmuls
        # start as early as possible.
        f1_bands = [0]
        while f1_bands[-1] < NBY:
            f1_bands.append(min(NBY, f1_bands[-1] + 3))
        f2_rows = [0]
        while f2_rows[-1] < ROWS2:
            f2_rows.append(min(ROWS2, f2_rows[-1] + 24))

        state = {"f1": 0, "f2": 0}

        def ensure_f1(band_needed):
            while (state["f1"] < len(f1_bands) - 1
                   and f1_bands[state["f1"]] <= min(band_needed, NBY - 1)):
                g = state["f1"]
                c0 = f1_bands[g] * NBX * 128
                c1 = f1_bands[g + 1] * NBX * 128
                nc.gpsimd.dma_start(out=f1b[:, c0:c1], in_=f1[:, c0:c1])
                state["f1"] += 1

        def ensure_f2(row_needed):
            while (state["f2"] < len(f2_rows) - 1
                   and f2_rows[state["f2"]] <= min(row_needed, ROWS2 - 1)):
                g = state["f2"]
                r0, r1 = f2_rows[g], f2_rows[g + 1]
                nc.gpsimd.dma_start(out=f2n[:, r0:r1], in_=f2[:, r0:r1])
                state["f2"] += 1

        ensure_f1(LA)
        ensure_f2((LA + 1) * PY + HY - 1)

        # pre-warm the PE clock (HAM): ~4 us of dummy matmuls into the
        # first PSUM pair while the first load chunks land, so the real
        # stream starts at the warm 2.4 GHz clock.
        warm = persist.tile([C, 512], BF16)
        nc.vector.memset(warm, 0.0)
        for _ in range(4):
            pw = psum_a.tile([128, 2, 512], F32, tag="ps")
            nc.tensor.matmul(pw[:, 0, :512], warm[:, :128], warm,
                             start=True, stop=True)
            nc.tensor.matmul(pw[:, 1, :512], warm[:, :128], warm,
                             start=True, stop=True)

        rot = 0
        for by in range(NBY):
            # prefetch ahead so the PE never starves (keeps HAM warm)
            ensure_f1(by + LA)
            ensure_f2((by + LA) * PY + HY - 1)
            sm = smpool.tile([128, NBX, NHALO], BF16)
            for bx0 in range(0, NBX, 2):
                pm = psum_a.tile([128, 2, 512], F32, tag="ps")
                for j in range(2):
                    bx = bx0 + j
                    blk = by * NBX + bx
                    lhsT = f1b[:, blk * 128:(blk + 1) * 128]
                    rhs = f2n[:, by * PY:by * PY + HY,
                              bx * PX:bx * PX + HX]
                    nc.tensor.matmul(pm[:, j, :NHALO], lhsT, rhs,
                                     start=True, stop=True)
                dst = sm[:, bx0:bx0 + 2, :]
                if rot == 0:
                    nc.vector.tensor_copy(out=dst, in_=pm[:, :, :NHALO])
                else:
                    nc.scalar.copy(out=dst, in_=pm[:, :, :NHALO])
                rot = (rot + 1) % 2
            nc.sync.dma_start(
                out=out[:, by * NBX * NHALO:(by + 1) * NBX * NHALO], in_=sm)


def _get_program():
    if "nc" not in _compiled:
        nc = bacc.Bacc("TRN2", target_bir_lowering=False, debug=False)
        f1 = nc.dram_tensor("f1", [C, N1], BF16,
                            kind="ExternalInput").ap()
        f2 = nc.dram_tensor("f2", [C, ROWS2, W2], BF16,
                            kind="ExternalInput").ap()
        out = nc.dram_tensor("tiles", [128, NBLK * NHALO], BF16,
                             kind="ExternalOutput").ap()
        _build_kernel(nc, f1, f2, out)
        nc.compile()
        _compiled["nc"] = nc
    return _compiled["nc"]


def _host_extract(tiles, inv1p, inv2p):
    """Sheared raw tiles [NBLK, 128, 384] + inv-norm planes ->
    [81, ROWS, WIDTH] normalized (fp32)."""
    v = tiles.reshape(NBY, NBX, PY, PX, HY, HX)
    out = np.empty((81, ROWS, WIDTH), np.float32)
    iy = np.arange(PY)[:, None]
    ix = np.arange(PX)[None, :]
    for dy in range(-4, 5):
        a = 4 - dy
        for dx in range(-4, 5):
            b = 4 - dx
            k = (dy + 4) * 9 + (dx + 4)
            g = v[:, :, iy, ix, iy + a, ix + b]      # [NBY, NBX, PY, PX]
            out[k] = (g.transpose(0, 2, 1, 3).reshape(ROWS, WIDTH)
                      * inv2p[a:a + ROWS, b:b + WIDTH])
    out *= inv1p[None]
    return out


def run_cores(in_maps, **kwargs):
    """Compile once and run the SPMD kernel on cores 0-7.

    Retries once: a freshly loaded NEFF occasionally hits a transient
    NRT exec-unit error right after a profiled session; the runtime
    recovers on the next execution.
    """
    import time

    nc = _get_program()
    try:
        return run_bass_kernel_spmd(nc, in_maps, core_ids=list(range(8)),
                                    **kwargs)
    except Exception:
        try:
            import jax.extend as jex

            jex.backend.clear_backends()
        except Exception:
            pass
        time.sleep(2.0)
        return run_bass_kernel_spmd(nc, in_maps, core_ids=list(range(8)),
                                    **kwargs)


def make_in_maps(feat1, feat2):
    feat1 = np.asarray(feat1, dtype=np.float32).astype(ml_dtypes.bfloat16)
    feat2 = np.asarray(feat2, dtype=np.float32).astype(ml_dtypes.bfloat16)
    in_maps = []
    for b in range(B):
        f2p = np.zeros((C, H + 8, W + 8), ml_dtypes.bfloat16)
        f2p[:, 4:-4, 4:-4] = feat2[b]
        for h in range(2):
            x0 = WIDTH * h
            # f1 block-major: [C, NBY, PY, NBX, PX] -> [C, NBY, NBX, PY, PX]
            f1s = feat1[b, :, :, x0:x0 + WIDTH].reshape(C, NBY, PY, NBX, PX)
            f1s = f1s.transpose(0, 1, 3, 2, 4).reshape(C, N1)
            in_maps.append({
                "f1": np.ascontiguousarray(f1s),
                "f2": np.ascontiguousarray(f2p[:, :, x0:x0 + WIDTH + 8]),
            })
    return in_maps


def _host_invnorms(feat1, feat2):
    """fp32 inverse L2 norms over C: inv1 [B, H, W]; inv2 padded
    [B, H+8, W+8] (zeros outside the frame -> raw dots there are 0)."""
    f1 = np.asarray(feat1, dtype=np.float32)
    f2 = np.asarray(feat2, dtype=np.float32)
    n1 = np.sqrt(np.einsum("bchw,bchw->bhw", f1, f1, optimize=True))
    n2 = np.sqrt(np.einsum("bchw,bchw->bhw", f2, f2, optimize=True))
    inv1 = 1.0 / np.maximum(n1, 1e-12)
    inv2 = np.zeros((B, H + 8, W + 8), np.float32)
    inv2[:, 4:-4, 4:-4] = 1.0 / np.maximum(n2, 1e-12)
    return inv1, inv2


def assemble(results, inv1, inv2):
    out = np.empty((B, 81, H, W), np.float32)
    for i, res in enumerate(results):
        tiles = np.asarray(res["tiles"]).astype(np.float32)
        tiles = tiles.reshape(128, NBLK, NHALO).transpose(1, 0, 2)
        b, h = i // 2, i % 2
        x0 = WIDTH * h
        out[b, :, :, x0:x0 + WIDTH] = _host_extract(
            tiles, inv1[b, :, x0:x0 + WIDTH],
            inv2[b, :, x0:x0 + WIDTH + 8])
    return out


def kernel(feat1, feat2):
    in_maps = make_in_maps(feat1, feat2)
    inv1, inv2 = _host_invnorms(feat1, feat2)
    res = run_cores(in_maps)
    return assemble(res.results, inv1, inv2)
